# revision 22
# baseline (speedup 1.0000x reference)
"""Trainium2 Bass kernel for quantized int8 3x3 conv (Conv2dQInt8).

Reference semantics (jax):
    x = (inputVec.f32 - 7) * 0.01          # [N=64, Cin=16, 256, 256]
    w = (weight.f32 - 3) * 0.01            # [Cout=16, Cin=16, 3, 3]
    b = clip(round(bias / 1e-4)) * 1e-4    # [16]
    out = conv_valid(x, w) + b             # [64, 16, 254, 254] fp32

Strategy:
  - Data-parallel over batch: 8 images per NeuronCore x 8 cores.
  - All arithmetic is exact: integer-valued bf16 operands (|x|<=128,
    |w-3|<=131 both exact in bf16), products < 2^16, accumulation of
    144 terms < 2^24 in fp32 PSUM -> the integer conv is exact; the
    1e-4 scale + dequantized bias are applied in fp32 at PSUM drain.
  - Conv as banded matmul: contraction K = (ci, r) = 16*8 = 128 rows of
    the image; stationary lhsT[128, 96] has band structure so M =
    (dh, co) = 6*16 = 96 produces 6 output rows x 16 channels at once.
    The 3 kw taps are 3 PSUM-accumulated matmuls over w-shifted views
    of the same rhs.  rhs free dim = 2 images x 254 cols = 508.
  - Input is shipped as int8 (4x less HBM traffic), cast to bf16
    on-chip (split across ScalarE and VectorE).  Both quantization
    zero-points are folded into the bias: conv(x-7, w-3) =
    conv(x, w-3) - 7*S[co], S[co] = sum(w[co]-3).
"""

import os
import sys

import numpy as np

sys.path.insert(0, "/opt/trn_rl_repo")

import ml_dtypes  # noqa: E402

N_CORES = 8
N_PER = 8  # images per core
CIN = 16
COUT = 16
H = W = 256
HO = WO = 254
DH = 6  # output rows per group
R = 8  # input rows per group (DH + 2)
# Row-group bases: 0,6,...,246 cover output rows 0..251; the tail group at
# 248 re-computes rows 248..251 and contributes rows 252..253 (dh=4,5 only).
GROUP_BASES = list(range(0, 252, 6)) + [248]
N_PAIRS = N_PER // 2

IN_ZP, W_ZP = 7, 3
OUT_SCALE = np.float32(1e-4)  # IN_SCALE * W_SCALE
B_SCALE = np.float32(1e-4)
INT32_MIN, INT32_MAX = -2147483648.0, 2147483647.0

_CACHE = {}


def _build_program():
    import concourse.tile as tile
    from concourse import bacc, mybir
    from contextlib import ExitStack

    AF = mybir.ActivationFunctionType
    ALU = mybir.AluOpType

    nc = bacc.Bacc(
        "TRN2", target_bir_lowering=False, debug=False, num_devices=N_CORES
    )
    # Host-side layouts chosen so every DMA collapses to <=3 AP dims:
    #   x_dev[ci, h, img, w]  -> group slice is [16, (r img w)] contiguous
    #   y_dev[h, co, img, w]  -> group slice is [(dh co), (img w)] contiguous
    x = nc.dram_tensor(
        "x", [CIN, H, N_PER, W], mybir.dt.int8, kind="ExternalInput"
    ).ap()
    wb = nc.dram_tensor(
        "wb", [128, 3, 96], mybir.dt.bfloat16, kind="ExternalInput"
    ).ap()
    bias = nc.dram_tensor(
        "bias", [96, 1], mybir.dt.float32, kind="ExternalInput"
    ).ap()
    y = nc.dram_tensor(
        "y", [HO, COUT, N_PER, WO], mybir.dt.float32, kind="ExternalOutput"
    ).ap()

    with tile.TileContext(nc) as tc, ExitStack() as ctx:
        const_pool = ctx.enter_context(tc.tile_pool(name="const", bufs=1))
        in_pool = ctx.enter_context(tc.tile_pool(name="inp", bufs=8))
        xb_pool = ctx.enter_context(tc.tile_pool(name="xb", bufs=8))
        ob_pool = ctx.enter_context(tc.tile_pool(name="ob", bufs=6))
        ps_pool = ctx.enter_context(tc.tile_pool(name="ps", bufs=2, space="PSUM"))

        wt = const_pool.tile([128, 3, 96], mybir.dt.bfloat16)
        nc.sync.dma_start(wt[:], wb)
        bt = const_pool.tile([96, 1], mybir.dt.float32)
        nc.sync.dma_start(bt[:], bias)

        NG = len(GROUP_BASES)
        LAG = 4  # software pipeline: load/cast runs LAG groups ahead of compute
        xb_tiles = {}
        for i in range(NG + LAG):
            if i < NG:
                r0 = GROUP_BASES[i]
                # [128=(ci,r), img, w] int8 tile <- rows r0..r0+7, all 8 images
                in_t = in_pool.tile([128, N_PER, W], mybir.dt.int8, tag="in")
                nc.gpsimd.dma_start(in_t[:], x[:, r0 : r0 + R, :, :])
                # cast int8 -> bf16, split between ScalarE and VectorE
                xb = xb_pool.tile([128, N_PER, W], mybir.dt.bfloat16, tag="xb")
                nc.scalar.activation(xb[:, 0:4, :], in_t[:, 0:4, :], AF.Copy)
                nc.vector.tensor_copy(out=xb[:, 4:8, :], in_=in_t[:, 4:8, :])
                xb_tiles[i] = xb
            if i < LAG:
                continue
            g = i - LAG
            r0 = GROUP_BASES[g]
            tail = r0 == 248
            xb = xb_tiles.pop(g)

            # one PSUM bank per image-pair
            ob = ob_pool.tile([96, N_PAIRS, 2, WO], mybir.dt.float32, tag="ob")
            for p in range(N_PAIRS):
                ps = ps_pool.tile([96, 2, WO], mybir.dt.float32, tag=f"ps{p}")
                for kwi in range(3):
                    nc.tensor.matmul(
                        ps[:],
                        wt[:, kwi, :],
                        xb[:, 2 * p : 2 * p + 2, kwi : kwi + WO],
                        start=(kwi == 0),
                        stop=(kwi == 2),
                    )
                # drain: out = psum * 1e-4 + bias_eff[co]  (fp32), split ACT/DVE
                if p < 2:
                    nc.scalar.activation(
                        ob[:, p], ps[:], AF.Identity,
                        bias=bt[:], scale=float(OUT_SCALE),
                    )
                else:
                    nc.vector.tensor_scalar(
                        ob[:, p], ps[:], float(OUT_SCALE), bt[:],
                        ALU.mult, ALU.add,
                    )
            # two ~390KB stores per group on the two HWDGE rings; each half
            # only depends on its own drain engine (ACT: imgs 0:4, DVE: 4:8)
            if tail:
                nc.sync.dma_start(y[252:254, :, 0:4, :], ob[64:96, 0:2])
                nc.scalar.dma_start(y[252:254, :, 4:8, :], ob[64:96, 2:4])
            else:
                nc.sync.dma_start(y[r0 : r0 + DH, :, 0:4, :], ob[:, 0:2])
                nc.scalar.dma_start(y[r0 : r0 + DH, :, 4:8, :], ob[:, 2:4])
    nc.compile()
    return nc


def _get_program():
    if "nc" not in _CACHE:
        _CACHE["nc"] = _build_program()
    return _CACHE["nc"]


def _host_weights(weight_np, bias_np):
    """Build the banded lhsT [128=(ci,r), 3=kw, 96=(dh,co)] and effective bias."""
    wq = weight_np.astype(np.float32) - W_ZP  # [co, ci, kh, kw]
    band = np.zeros((CIN, R, 3, DH, COUT), np.float32)
    for dh in range(DH):
        for kh in range(3):
            # band[ci, dh+kh, kw, dh, co] = wq[co, ci, kh, kw]
            band[:, dh + kh, :, dh, :] = wq[:, :, kh, :].transpose(1, 2, 0)
    wband = band.reshape(128, 3, 96).astype(ml_dtypes.bfloat16)

    # dequantized bias, computed in fp32 exactly like the reference
    b32 = bias_np.astype(np.float32)
    q = np.round(b32 / B_SCALE)
    q = np.clip(q, INT32_MIN, INT32_MAX).astype(np.float32)
    b_dq = q * B_SCALE  # fp32
    s_co = wq.sum(axis=(1, 2, 3))  # S[co] = sum(w - 3)
    bias_eff = b_dq - np.float32(IN_ZP) * OUT_SCALE * s_co  # [16] fp32
    bias96 = np.tile(bias_eff.astype(np.float32), DH)[:, None]  # [(dh,co)=96, 1]
    return wband, np.ascontiguousarray(bias96, np.float32)


def _run(inputVec, weight, bias, trace=False):
    from concourse.bass_utils import run_bass_kernel_spmd

    x_np = np.asarray(inputVec)
    w_np = np.asarray(weight)
    b_np = np.asarray(bias)
    assert x_np.shape == (N_CORES * N_PER, CIN, H, W), x_np.shape

    x8 = x_np.astype(np.int8)  # values are in [-128, 127]
    wband, bias96 = _host_weights(w_np, b_np)

    nc = _get_program()
    in_maps = []
    for c in range(N_CORES):
        shard = x8[c * N_PER : (c + 1) * N_PER]  # [img, ci, h, w]
        shard = np.ascontiguousarray(shard.transpose(1, 2, 0, 3))  # [ci,h,img,w]
        in_maps.append({"x": shard, "wb": wband, "bias": bias96})
    res = run_bass_kernel_spmd(
        nc, in_maps, core_ids=list(range(N_CORES)), trace=trace
    )
    # y_dev is [h, co, img, w] -> [img, co, h, w], then stack shards on batch
    out = np.concatenate(
        [res.results[c]["y"].transpose(2, 1, 0, 3) for c in range(N_CORES)],
        axis=0,
    )
    return np.ascontiguousarray(out, dtype=np.float32), res


def kernel(inputVec, weight, bias, groups=1, **_ignored):
    assert int(np.asarray(groups)) == 1
    out, _ = _run(inputVec, weight, bias, trace=False)
    return out


def kernel_profiled(inputVec, weight, bias, groups=1):
    out, res = _run(inputVec, weight, bias, trace=True)
    return out, res


# revision 23
# speedup vs baseline: 1.0172x; 1.0172x over previous
"""Trainium2 Bass kernel for quantized int8 3x3 conv (Conv2dQInt8).

Reference semantics (jax):
    x = (inputVec.f32 - 7) * 0.01          # [N=64, Cin=16, 256, 256]
    w = (weight.f32 - 3) * 0.01            # [Cout=16, Cin=16, 3, 3]
    b = clip(round(bias / 1e-4)) * 1e-4    # [16]
    out = conv_valid(x, w) + b             # [64, 16, 254, 254] fp32

Strategy:
  - Data-parallel over batch: 8 images per NeuronCore x 8 cores.
  - All arithmetic is exact: integer-valued bf16 operands (|x|<=128,
    |w-3|<=131 both exact in bf16), products < 2^16, accumulation of
    144 terms < 2^24 in fp32 PSUM -> the integer conv is exact; the
    1e-4 scale + dequantized bias are applied in fp32 at PSUM drain.
  - Conv as banded matmul: contraction K = (ci, r) = 16*8 = 128 rows of
    the image; stationary lhsT[128, 96] has band structure so M =
    (dh, co) = 6*16 = 96 produces 6 output rows x 16 channels at once.
    The 3 kw taps are 3 PSUM-accumulated matmuls over w-shifted views
    of the same rhs.  rhs free dim = 2 images x 254 cols = 508.
  - Input is shipped as int8 (4x less HBM traffic), cast to bf16
    on-chip (split across ScalarE and VectorE).  Both quantization
    zero-points are folded into the bias: conv(x-7, w-3) =
    conv(x, w-3) - 7*S[co], S[co] = sum(w[co]-3).
"""

import os
import sys

import numpy as np

sys.path.insert(0, "/opt/trn_rl_repo")

import ml_dtypes  # noqa: E402

N_CORES = 8
N_PER = 8  # images per core
CIN = 16
COUT = 16
H = W = 256
HO = WO = 254
DH = 6  # output rows per group
R = 8  # input rows per group (DH + 2)
# Row-group bases: 0,6,...,246 cover output rows 0..251; the tail group at
# 248 re-computes rows 248..251 and contributes rows 252..253 (dh=4,5 only).
GROUP_BASES = list(range(0, 252, 6)) + [248]
N_PAIRS = N_PER // 2

IN_ZP, W_ZP = 7, 3
OUT_SCALE = np.float32(1e-4)  # IN_SCALE * W_SCALE
B_SCALE = np.float32(1e-4)
INT32_MIN, INT32_MAX = -2147483648.0, 2147483647.0

_CACHE = {}


def _build_program():
    import concourse.tile as tile
    from concourse import bacc, mybir
    from contextlib import ExitStack

    AF = mybir.ActivationFunctionType
    ALU = mybir.AluOpType

    nc = bacc.Bacc(
        "TRN2", target_bir_lowering=False, debug=False, num_devices=N_CORES
    )
    # Host-side layouts chosen so every DMA collapses to <=3 AP dims:
    #   x_dev[ci, h, img, w]  -> group slice is [16, (r img w)] contiguous
    #   y_dev[h, co, img, w]  -> group slice is [(dh co), (img w)] contiguous
    x = nc.dram_tensor(
        "x", [CIN, H, N_PER, W], mybir.dt.int8, kind="ExternalInput"
    ).ap()
    wb = nc.dram_tensor(
        "wb", [128, 3, 96], mybir.dt.bfloat16, kind="ExternalInput"
    ).ap()
    bias = nc.dram_tensor(
        "bias", [96, 1], mybir.dt.float32, kind="ExternalInput"
    ).ap()
    y = nc.dram_tensor(
        "y", [HO, COUT, N_PER, WO], mybir.dt.float32, kind="ExternalOutput"
    ).ap()

    with tile.TileContext(nc) as tc, ExitStack() as ctx:
        const_pool = ctx.enter_context(tc.tile_pool(name="const", bufs=1))
        in_pool = ctx.enter_context(tc.tile_pool(name="inp", bufs=8))
        xb_pool = ctx.enter_context(tc.tile_pool(name="xb", bufs=8))
        ob_pool = ctx.enter_context(tc.tile_pool(name="ob", bufs=6))
        ps_pool = ctx.enter_context(tc.tile_pool(name="ps", bufs=2, space="PSUM"))

        wt = const_pool.tile([128, 3, 96], mybir.dt.bfloat16)
        nc.sync.dma_start(wt[:], wb)
        bt = const_pool.tile([96, 1], mybir.dt.float32)
        nc.sync.dma_start(bt[:], bias)

        NG = len(GROUP_BASES)
        LAG = 3  # software pipeline: load/cast runs LAG groups ahead of compute
        xb_tiles = {}
        for i in range(NG + LAG):
            if i < NG:
                r0 = GROUP_BASES[i]
                # [128=(ci,r), img, w] int8 tile <- rows r0..r0+7, all 8 images
                in_t = in_pool.tile([128, N_PER, W], mybir.dt.int8, tag="in")
                nc.gpsimd.dma_start(in_t[:], x[:, r0 : r0 + R, :, :])
                # cast int8 -> bf16, split between ScalarE and VectorE
                xb = xb_pool.tile([128, N_PER, W], mybir.dt.bfloat16, tag="xb")
                nc.scalar.activation(xb[:, 0:4, :], in_t[:, 0:4, :], AF.Copy)
                nc.vector.tensor_copy(out=xb[:, 4:8, :], in_=in_t[:, 4:8, :])
                xb_tiles[i] = xb
            if i < LAG:
                continue
            g = i - LAG
            r0 = GROUP_BASES[g]
            tail = r0 == 248
            xb = xb_tiles.pop(g)

            # one PSUM bank per image-pair
            ob = ob_pool.tile([96, N_PAIRS, 2, WO], mybir.dt.float32, tag="ob")
            for p in range(N_PAIRS):
                ps = ps_pool.tile([96, 2, WO], mybir.dt.float32, tag=f"ps{p}")
                for kwi in range(3):
                    nc.tensor.matmul(
                        ps[:],
                        wt[:, kwi, :],
                        xb[:, 2 * p : 2 * p + 2, kwi : kwi + WO],
                        start=(kwi == 0),
                        stop=(kwi == 2),
                    )
                # drain: out = psum * 1e-4 + bias_eff[co]  (fp32), split ACT/DVE
                if p < 2:
                    nc.scalar.activation(
                        ob[:, p], ps[:], AF.Identity,
                        bias=bt[:], scale=float(OUT_SCALE),
                    )
                else:
                    nc.vector.tensor_scalar(
                        ob[:, p], ps[:], float(OUT_SCALE), bt[:],
                        ALU.mult, ALU.add,
                    )
            # two ~390KB stores per group on the two HWDGE rings; each half
            # only depends on its own drain engine (ACT: imgs 0:4, DVE: 4:8)
            if tail:
                nc.sync.dma_start(y[252:254, :, 0:4, :], ob[64:96, 0:2])
                nc.scalar.dma_start(y[252:254, :, 4:8, :], ob[64:96, 2:4])
            else:
                nc.sync.dma_start(y[r0 : r0 + DH, :, 0:4, :], ob[:, 0:2])
                nc.scalar.dma_start(y[r0 : r0 + DH, :, 4:8, :], ob[:, 2:4])
    nc.compile()
    return nc


def _get_program():
    if "nc" not in _CACHE:
        _CACHE["nc"] = _build_program()
    return _CACHE["nc"]


def _host_weights(weight_np, bias_np):
    """Build the banded lhsT [128=(ci,r), 3=kw, 96=(dh,co)] and effective bias."""
    wq = weight_np.astype(np.float32) - W_ZP  # [co, ci, kh, kw]
    band = np.zeros((CIN, R, 3, DH, COUT), np.float32)
    for dh in range(DH):
        for kh in range(3):
            # band[ci, dh+kh, kw, dh, co] = wq[co, ci, kh, kw]
            band[:, dh + kh, :, dh, :] = wq[:, :, kh, :].transpose(1, 2, 0)
    wband = band.reshape(128, 3, 96).astype(ml_dtypes.bfloat16)

    # dequantized bias, computed in fp32 exactly like the reference
    b32 = bias_np.astype(np.float32)
    q = np.round(b32 / B_SCALE)
    q = np.clip(q, INT32_MIN, INT32_MAX).astype(np.float32)
    b_dq = q * B_SCALE  # fp32
    s_co = wq.sum(axis=(1, 2, 3))  # S[co] = sum(w - 3)
    bias_eff = b_dq - np.float32(IN_ZP) * OUT_SCALE * s_co  # [16] fp32
    bias96 = np.tile(bias_eff.astype(np.float32), DH)[:, None]  # [(dh,co)=96, 1]
    return wband, np.ascontiguousarray(bias96, np.float32)


def _run(inputVec, weight, bias, trace=False):
    from concourse.bass_utils import run_bass_kernel_spmd

    x_np = np.asarray(inputVec)
    w_np = np.asarray(weight)
    b_np = np.asarray(bias)
    assert x_np.shape == (N_CORES * N_PER, CIN, H, W), x_np.shape

    x8 = x_np.astype(np.int8)  # values are in [-128, 127]
    wband, bias96 = _host_weights(w_np, b_np)

    nc = _get_program()
    in_maps = []
    for c in range(N_CORES):
        shard = x8[c * N_PER : (c + 1) * N_PER]  # [img, ci, h, w]
        shard = np.ascontiguousarray(shard.transpose(1, 2, 0, 3))  # [ci,h,img,w]
        in_maps.append({"x": shard, "wb": wband, "bias": bias96})
    res = run_bass_kernel_spmd(
        nc, in_maps, core_ids=list(range(N_CORES)), trace=trace
    )
    # y_dev is [h, co, img, w] -> [img, co, h, w], then stack shards on batch
    out = np.concatenate(
        [res.results[c]["y"].transpose(2, 1, 0, 3) for c in range(N_CORES)],
        axis=0,
    )
    return np.ascontiguousarray(out, dtype=np.float32), res


def kernel(inputVec, weight, bias, groups=1, **_ignored):
    assert int(np.asarray(groups)) == 1
    out, _ = _run(inputVec, weight, bias, trace=False)
    return out


def kernel_profiled(inputVec, weight, bias, groups=1):
    out, res = _run(inputVec, weight, bias, trace=True)
    return out, res
